# revision 31
# baseline (speedup 1.0000x reference)
"""Causal multi-head attention for TRN2, sharded across 8 NeuronCores.

Problem: x[4,2048,1024] -> 16-head causal self-attention (head_dim 64) with
QKV + output projections, fp32.

Sharding: core c -> batch b = c // 2, head-group g = c % 2 (heads g*8..g*8+7).
Per core: Q/K/V projections use the 512 weight columns of its head-group
(column-parallel); attention runs over its 8 heads; the output projection
uses the matching 512 rows of wo (row-parallel), so each core emits a
partial [2048,1024] output and the host sums the two partials per batch.
bo is added on the g==0 cores only (g==1 cores receive zeros).

v2 over the 263.7us baseline: paired-head row-tiled score matmuls.
The two heads of a QK column block t live in disjoint partition halves
(even head: features on partitions 0:64, odd head: 64:128), so per k-block
the two heads' score matmuls are K=64 row-tiled to array rows 0:63 /
64:127 and run CONCURRENTLY (~2x score throughput), with no extra
copies and no zero-padding of the K contraction. One ACTIVATE covers
both heads' PSUM banks per k-block (same ACT count as the baseline's
kb-pair exp, but with zero junk columns: both slots share the same
causal clip). AV accumulates per head as before; the A PSUM bank is
released early via a [65,512] PSUM->SBUF copy (replaces the baseline's
row-64 sums copy at the same DVE cost), so the 2-bank A pool sustains
back-to-back head pairs.

Retained from the baseline: fp8e4 DoubleRow Q/K projections with
host-scaled x32 weights (score scale folded into the exp scale 2^-13),
bk dropped (softmax-invariant), bq folded into the Q PSUM->SBUF move,
j-chunk-major host layouts, dep-free PE warmup matmuls, bf16 V with a
ones column computing softmax denominators inside the AV matmul, and
the chunk-pipelined filler schedule (chunk j attention interleaved with
chunk j+1 projections and chunk j-1 output projection).

Measured NOT to help (kept out): fp8 E/V/O anywhere in the value path
(numpy-sim rel err 2.8e-2..4e-2 vs the 2e-2 budget); col-tiled AV pairs
(no room for the denominator ones column in 128 array columns).
"""

import os
from contextlib import ExitStack

import numpy as np

import concourse.bacc as bacc
import concourse.mybir as mybir
import concourse.tile as tile
from concourse.bass_utils import run_bass_kernel_spmd
from concourse.masks import make_upper_triangular

F32 = mybir.dt.float32
BF16 = mybir.dt.bfloat16
F8 = mybir.dt.float8e4
AF = mybir.ActivationFunctionType
ALU = mybir.AluOpType
DR = mybir.MatmulPerfMode.DoubleRow

B = 4
S = 2048
D = 1024
HD = 64
HG = 8  # heads per core
QC = HG * HD  # 512 local q/k/v columns
N_CORES = 8
WSCALE = 32.0  # host scale on wq/wk so fp8 stays in normal range
SC = 0.125 / (WSCALE * WSCALE)  # exp scale: 1/sqrt(HD) / (32*32) = 2^-13

_NC_CACHE = {}
LAST_RESULT = None  # BassKernelResults of the most recent kernel() call


def _build_nc(s: int = S, num_devices: int = N_CORES):
    P = 128
    NQ = s // 512  # 512-col q-chunks
    NS = s // P  # 128-row s-tiles
    ND = D // P  # bf16 contraction tiles
    NC = D // 256  # fp8 DoubleRow contraction tiles
    NT = QC // P  # 128-row tiles of the local q/k/v columns
    VW = HD + 1  # 65: per-head V block width (64 cols + ones col)
    VPAD = 7 * VW + P  # 583: last head's lhsT slice must fit

    nc = bacc.Bacc("TRN2", target_bir_lowering=False, debug=False, num_devices=num_devices)

    xf8_d = nc.dram_tensor("xf8", [P, NQ * NC * 2 * 512], F8, kind="ExternalInput").ap()
    xbf_d = nc.dram_tensor("xbf", [P, NQ * ND * 512], BF16, kind="ExternalInput").ap()
    wqf8_d = nc.dram_tensor("wqf8", [P, NC * 2 * QC], F8, kind="ExternalInput").ap()
    wkf8_d = nc.dram_tensor("wkf8", [P, NC * 2 * QC], F8, kind="ExternalInput").ap()
    wv_d = nc.dram_tensor("wv", [P, ND * QC], BF16, kind="ExternalInput").ap()
    wo_d = nc.dram_tensor("wo", [P, NT * D], BF16, kind="ExternalInput").ap()
    bq32_d = nc.dram_tensor("bq32", [QC], F32, kind="ExternalInput").ap()
    bvb_d = nc.dram_tensor("bvb", [P, QC], F32, kind="ExternalInput").ap()
    bob_d = nc.dram_tensor("bob", [P, D], F32, kind="ExternalInput").ap()
    out_d = nc.dram_tensor("out", [s, D], F32, kind="ExternalOutput").ap()

    xf8_r = xf8_d.rearrange("p (j c i s) -> p j c i s", j=NQ, c=NC, i=2)
    xbf_r = xbf_d.rearrange("p (j d s) -> p j d s", j=NQ, d=ND)

    with tile.TileContext(nc) as tc:
        with ExitStack() as ctx:
            consts = ctx.enter_context(tc.tile_pool(name="consts", bufs=1))
            persist = ctx.enter_context(tc.tile_pool(name="persist", bufs=1))
            e_pool = ctx.enter_context(tc.tile_pool(name="epool", bufs=4))
            n_pool = ctx.enter_context(tc.tile_pool(name="npool", bufs=4))
            b_pool = ctx.enter_context(tc.tile_pool(name="bpool", bufs=4))
            acc_pool = ctx.enter_context(tc.tile_pool(name="accpool", bufs=4))
            o_pool = ctx.enter_context(tc.tile_pool(name="opool", bufs=3))
            proj_psum = ctx.enter_context(tc.tile_pool(name="proj_ps", bufs=2, space="PSUM"))
            s_psum = ctx.enter_context(tc.tile_pool(name="s_ps", bufs=2, space="PSUM"))
            a_psum = ctx.enter_context(tc.tile_pool(name="a_ps", bufs=2, space="PSUM"))

            # --- dep-free PE warmup first: matmuls on a memset junk tile
            # open the HAM clock gate while the DMAs stream in
            junk = consts.tile([P, P], BF16)
            nc.gpsimd.memset(junk[:], 0.5)
            warm = s_psum.tile([P, 2, 512], F32, tag="s", name="warm")
            for _ in range(52):
                nc.tensor.matmul(
                    warm[:, 0, 0:P], lhsT=junk[:], rhs=junk[:], start=True, stop=True
                )

            QT = persist.tile([P, NT, s], BF16)
            KT2 = persist.tile([P, NT, s], BF16)  # even head on 0:64, odd on 64:128
            V = persist.tile([P, NS, VPAD + 1], BF16)
            AT = persist.tile([P, NT, s], BF16)

            # V ones columns (denominator trick); V's tail padding stays junk
            # (it only feeds never-read PSUM rows 65+ of the AV accumulators)
            nc.gpsimd.memset(
                V[:, :, 0 : HG * VW].rearrange("p s (h c) -> p s h c", c=VW)[:, :, :, HD : HD + 1],
                1.0,
            )

            tri = consts.tile([P, P], F32)
            make_upper_triangular(nc, tri[:], val=1.0, diag=True)
            tri_b = consts.tile([P, P], BF16)
            nc.vector.tensor_copy(tri_b[:], tri[:])

            # --- inputs in order of first use; DMA issue is ~0.7us each on
            # the sync queue so first-needed must go first
            wqf8_sb = persist.tile([P, NC, 2, QC], F8)
            wkf8_sb = persist.tile([P, NC, 2, QC], F8)
            xf8_sb = persist.tile([P, NQ, NC, 2, 512], F8)
            xbf_sb = persist.tile([P, NQ, ND, 512], BF16)
            wv_sb = persist.tile([P, ND, QC], BF16)
            wo_sb = persist.tile([P, NT, D], BF16)
            bqc = consts.tile([P, NT], F32)
            bvb = consts.tile([P, QC], F32)
            bob = consts.tile([P, D], F32)

            nc.sync.dma_start(wqf8_sb[:], wqf8_d.rearrange("p (c i m) -> p c i m", c=NC, i=2))
            nc.sync.dma_start(xf8_sb[:, 0], xf8_r[:, 0])
            nc.sync.dma_start(wkf8_sb[:], wkf8_d.rearrange("p (c i m) -> p c i m", c=NC, i=2))
            nc.sync.dma_start(bqc[:], bq32_d.rearrange("(t p) -> p t", p=P))
            nc.sync.dma_start(wv_sb[:], wv_d.rearrange("p (d m) -> p d m", d=ND))
            nc.sync.dma_start(xbf_sb[:, 0], xbf_r[:, 0])
            nc.sync.dma_start(bvb[:], bvb_d)
            nc.sync.dma_start(bob[:], bob_d)
            # chunk j's attention interleaves V(j+1) projections in its FIRST
            # filler slots, so xbf[j+1] must land before xf8[j+1]
            for j in range(1, NQ):
                nc.sync.dma_start(xbf_sb[:, j], xbf_r[:, j])
                nc.sync.dma_start(xf8_sb[:, j], xf8_r[:, j])
            nc.sync.dma_start(wo_sb[:], wo_d.rearrange("p (t e) -> p t e", t=NT))

            proj_open = {}  # g -> (ps tile, progress) for half-issued groups

            def proj_group(j, g, half=None):
                """One psum-group of the j-chunk projections; g in 0..11.

                half=0 issues the first half of the matmul chain, half=1 the
                rest + the PSUM read, so a filler slot never blocks the PE
                queue for more than ~4 matmuls. half=None does it all.
                """
                js = slice(j * 512, (j + 1) * 512)
                kind, t = divmod(g, NT)
                if half in (None, 0):
                    ps = proj_psum.tile([P, 512], F32, tag="pp", name="pp")
                    proj_open[(j, g)] = ps
                else:
                    ps = proj_open.pop((j, g))
                if kind == 0:  # Q (fp8 DoubleRow)
                    cr = {None: range(NC), 0: range(NC // 2), 1: range(NC // 2, NC)}[half]
                    for c in cr:
                        nc.tensor.matmul(
                            ps[:],
                            lhsT=wqf8_sb[:, c, :, t * P : (t + 1) * P],
                            rhs=xf8_sb[:, j, c],
                            start=(c == 0),
                            stop=(c == NC - 1),
                            perf_mode=DR,
                        )
                    if half != 0:
                        nc.vector.tensor_scalar_add(QT[:, t, js], ps[:], bqc[:, t : t + 1])
                elif kind == 1:  # K (fp8 DoubleRow)
                    cr = {None: range(NC), 0: range(NC // 2), 1: range(NC // 2, NC)}[half]
                    for c in cr:
                        nc.tensor.matmul(
                            ps[:],
                            lhsT=wkf8_sb[:, c, :, t * P : (t + 1) * P],
                            rhs=xf8_sb[:, j, c],
                            start=(c == 0),
                            stop=(c == NC - 1),
                            perf_mode=DR,
                        )
                    if half != 0:
                        nc.vector.tensor_copy(KT2[:, t, js], ps[:])
                else:  # V s-tile 4j+t (bf16)
                    st = 4 * j + t
                    dr = {None: range(ND), 0: range(ND // 2), 1: range(ND // 2, ND)}[half]
                    for d in dr:
                        nc.tensor.matmul(
                            ps[:],
                            lhsT=xbf_sb[:, j, d, t * P : (t + 1) * P],
                            rhs=wv_sb[:, d, :],
                            start=(d == 0),
                            stop=(d == ND - 1),
                        )
                    if half != 0:
                        dst = V[:, st, 0 : HG * VW].rearrange("p (h c) -> p h c", c=VW)[:, :, 0:HD]
                        src = ps.rearrange("p (h c) -> p h c", c=HD)
                        bsrc = bvb.rearrange("p (h c) -> p h c", c=HD)
                        nc.vector.tensor_tensor(dst, src, bsrc, ALU.add)

            def normalize(j, t, A_ps, pb, last=False):
                """Divide A by the denominator row and store into AT.

                The sums+acc copies free the A bank early so the 2-buf pool
                sustains back-to-back pairs; the last pair skips the acc copy
                (nothing reuses its banks) to shorten the kernel tail.
                """
                sums = n_pool.tile([1, 512], F32, tag="sums", name="sums")
                nc.vector.tensor_copy(sums[:], A_ps[HD : HD + 1, :])
                if last:
                    acc = A_ps
                else:
                    acc = acc_pool.tile([P, 512], F32, name="acc")
                    nc.vector.tensor_copy(acc[0:HD, :], A_ps[0:HD, :])
                rec = n_pool.tile([1, 512], F32, tag="rec", name="rec")
                nc.vector.reciprocal_approx_fast(rec[:], sums[:])
                bc = b_pool.tile([HD, 512], F32, name="bc")
                nc.gpsimd.partition_broadcast(bc[:], rec[0:1, :])
                nc.vector.tensor_tensor(
                    AT[pb : pb + HD, t, j * 512 : (j + 1) * 512],
                    acc[0:HD, :],
                    bc[:],
                    ALU.mult,
                )

            def attn_chunk(j, filler, last_chunk=False):
                """Attention for all 4 head pairs of q-chunk j as ONE
                continuous round stream (no pipeline drain at pair
                boundaries).

                Per k-block kb of pair t, the even head's score matmul
                (K=64, array rows 0:63) and the odd head's (rows 64:127)
                alternate so consecutive PE instructions target disjoint row
                groups and run concurrently. One exp covers both heads'
                banks. AV runs LAG rounds behind the scores so a blocked AV
                (its exp still running, or its tri-mask queued behind other
                DVE work) never head-of-line-blocks the next scores in the
                in-order PE queue and starves the saturated ACT engine.
                """
                nkb = 4 * j + 4
                LAG = 2
                rounds = [(t, kb) for t in range(NT) for kb in range(nkb)]
                rtot = len(rounds) + LAG
                nfill = len(filler)
                Es = {}
                A = {}
                for r in range(rtot):
                    if r < len(rounds):
                        t, kb = rounds[r]
                        y0 = max(0, P * (kb - 4 * j))
                        st = s_psum.tile([P, 2, 512], F32, tag="s", name="sp")
                        sp = st[:, :, y0:]
                        nc.tensor.matmul(
                            sp[:, 0],
                            lhsT=KT2[0:64, t, kb * P : (kb + 1) * P],
                            rhs=QT[0:64, t, j * 512 + y0 : (j + 1) * 512],
                            start=True,
                            stop=True,
                        )
                        nc.tensor.matmul(
                            sp[:, 1],
                            lhsT=KT2[64:128, t, kb * P : (kb + 1) * P],
                            rhs=QT[64:128, t, j * 512 + y0 : (j + 1) * 512],
                            start=True,
                            stop=True,
                        )
                        E = e_pool.tile([P, 2, 512], BF16, name="E")
                        nc.scalar.activation(E[:, :, y0:], sp, AF.Exp, scale=SC)
                        if kb >= 4 * j:  # diagonal block: causal triangle mask
                            nc.vector.tensor_tensor(
                                E[:, 0, y0 : y0 + P], E[:, 0, y0 : y0 + P], tri_b[:], ALU.mult
                            )
                            nc.vector.tensor_tensor(
                                E[:, 1, y0 : y0 + P], E[:, 1, y0 : y0 + P], tri_b[:], ALU.mult
                            )
                        Es[(t, kb)] = (E, y0)
                    if r >= LAG:
                        t, kb = rounds[r - LAG]
                        E, y0 = Es.pop((t, kb))
                        if kb == 0:
                            A[t] = (
                                a_psum.tile([P, 512], F32, tag="A", name="Ae"),
                                a_psum.tile([P, 512], F32, tag="A", name="Ao"),
                            )
                        for m, A_ps in enumerate(A[t]):
                            nc.tensor.matmul(
                                A_ps[:, y0:],
                                lhsT=V[:, kb, (2 * t + m) * VW : (2 * t + m) * VW + P],
                                rhs=E[:, m, y0:],
                                start=(kb == 0),
                                stop=(kb == nkb - 1),
                            )
                        if kb == nkb - 1:
                            last = last_chunk and t == NT - 1
                            A_e, A_o = A.pop(t)
                            normalize(j, t, A_e, 0, last=last)
                            normalize(j, t, A_o, 64, last=last)
                    k0 = (nfill * r) // rtot
                    k1 = (nfill * (r + 1)) // rtot
                    for kind, jj, g, hf in filler[k0:k1]:
                        if kind == "p":
                            proj_group(jj, g, half=hf)
                        else:
                            out_proj_group(jj, g, half=hf)

            out_open = {}

            def out_proj_group(j, g, half=None, use_s_pool=False):
                st = 4 * j + g // 2
                oc = g % 2
                if half in (None, 0):
                    if use_s_pool:  # tail: rotate through the idle score banks
                        o_ps = s_psum.tile([P, 2, 512], F32, tag="s", name="o_ps")[:, 0]
                    else:
                        o_ps = proj_psum.tile([P, 512], F32, tag="pp", name="o_ps")
                    out_open[(j, g)] = o_ps
                else:
                    o_ps = out_open.pop((j, g))
                tr = {None: range(NT), 0: range(NT // 2), 1: range(NT // 2, NT)}[half]
                for t2 in tr:
                    nc.tensor.matmul(
                        o_ps[:],
                        lhsT=AT[:, t2, st * P : (st + 1) * P],
                        rhs=wo_sb[:, t2, oc * 512 : (oc + 1) * 512],
                        start=(t2 == 0),
                        stop=(t2 == NT - 1),
                    )
                if half == 0:
                    return
                ot = o_pool.tile([P, 512], F32, name="ot")
                nc.vector.tensor_tensor(
                    ot[:], o_ps[:], bob[:, oc * 512 : (oc + 1) * 512], ALU.add
                )
                nc.sync.dma_start(
                    out_d[st * P : (st + 1) * P, oc * 512 : (oc + 1) * 512], ot[:]
                )

            def keep_warm(n):
                kw = s_psum.tile([P, 2, 512], F32, tag="s", name="kw")
                for _ in range(n):
                    nc.tensor.matmul(
                        kw[:, 0, 0:P], lhsT=junk[:], rhs=junk[:], start=True, stop=True
                    )

            # j-chunk 0: only pair-0's Q/K and the first V s-tile up front so
            # the first exp lands as early as possible; the rest of chunk 0's
            # projections ride attention(0)'s filler stream (V s-tile i is
            # first read by the AV of k-block i, one round after its scores).
            # Then pipeline: attention(j) interleaved with the projections of
            # chunk j+1 at k-block granularity, plus chunk j-1's output
            # projection. Groups are split into halves so one filler slot
            # never blocks the PE queue for more than ~4 matmuls.
            for g in (0, 4, 8):
                proj_group(0, g)
            for j in range(NQ):
                # V groups first: attn(j+1)'s first AVs need the new diagonal
                # V tiles, while K tile t only gates pair t there, so
                # emitting V,Q,K makes every dependency land before its use
                filler = []
                if j == 0:
                    filler += [("p", 0, g) for g in (9, 10, 11, 1, 5, 2, 6, 3, 7)]
                if j + 1 < NQ:
                    filler += [("p", j + 1, g) for g in (8, 9, 10, 11, 0, 1, 2, 3, 4, 5, 6, 7)]
                if j > 0:
                    filler += [("o", j - 1, g) for g in range(8)]
                filler = [(k, jj, g, hf) for k, jj, g in filler for hf in (0, 1)]
                attn_chunk(j, filler, last_chunk=(j == NQ - 1))
                if j == NQ - 1:
                    # bridge the last pair's normalize chain before the tail
                    # so the HAM clock gate stays open for the output proj
                    keep_warm(56)
            # tail: the last chunk's output projection, alternating psum
            # pools so the bias-add/DMA chain never gates the matmuls
            for g in range(8):
                out_proj_group(NQ - 1, g, half=None, use_s_pool=bool(g % 2))

    nc.compile()

    return nc


def _get_nc():
    if "nc" not in _NC_CACHE:
        _NC_CACHE["nc"] = _build_nc()
    return _NC_CACHE["nc"]


def make_in_maps(x, wq, bq, wk, bk, wv, bv, wo, bo, n_cores=N_CORES):
    import ml_dtypes

    bf = ml_dtypes.bfloat16
    f8 = ml_dtypes.float8_e4m3
    P = 128
    NQ = S // 512
    x = np.asarray(x, np.float32)
    wq, wk, wv, wo = (np.asarray(a, np.float32) for a in (wq, wk, wv, wo))
    bq = np.asarray(bq, np.float32)
    bv, bo = np.asarray(bv, np.float32), np.asarray(bo, np.float32)

    # per-batch x layouts (shared by both cores of the batch)
    xf8_b, xbf_b = [], []
    for b in range(B):
        xT = np.ascontiguousarray(x[b].T)  # [D, S]
        # [p, j, c, i, s] = xT[256c+128i+p, 512j+s]
        xf8 = (
            xT.reshape(4, 2, P, NQ, 512)
            .transpose(2, 3, 0, 1, 4)
            .reshape(P, -1)
            .astype(f8)
        )
        # [p, j, d, s] = xT[128d+p, 512j+s]
        xbf = (
            xT.reshape(8, P, NQ, 512).transpose(1, 2, 0, 3).reshape(P, -1).astype(bf)
        )
        xf8_b.append(np.ascontiguousarray(xf8))
        xbf_b.append(np.ascontiguousarray(xbf))

    def wqk_f8(w, cs):
        # [p, c, i, m] = WSCALE * w[256c+128i+p, cs][., m]
        return np.ascontiguousarray(
            (w[:, cs] * WSCALE).reshape(4, 2, P, QC).transpose(2, 0, 1, 3).reshape(P, -1).astype(f8)
        )

    in_maps = []
    for c in range(n_cores):
        b, g = c // 2, c % 2
        cs = slice(g * QC, (g + 1) * QC)
        wv_l = np.ascontiguousarray(
            wv[:, cs].reshape(8, P, QC).transpose(1, 0, 2).reshape(P, -1).astype(bf)
        )
        wo_l = np.ascontiguousarray(
            wo[cs, :].reshape(4, P, D).transpose(1, 0, 2).reshape(P, -1).astype(bf)
        )
        in_maps.append(
            {
                "xf8": xf8_b[b],
                "xbf": xbf_b[b],
                "wqf8": wqk_f8(wq, cs),
                "wkf8": wqk_f8(wk, cs),
                "wv": wv_l,
                "wo": wo_l,
                "bq32": np.ascontiguousarray(bq[cs] * WSCALE),
                "bvb": np.ascontiguousarray(np.broadcast_to(bv[cs], (128, QC))),
                "bob": np.ascontiguousarray(
                    np.broadcast_to(bo if g == 0 else np.zeros_like(bo), (128, D))
                ),
            }
        )
    return in_maps


def kernel(x, wq, bq, wk, bk, wv, bv, wo, bo):
    global LAST_RESULT
    in_maps = make_in_maps(x, wq, bq, wk, bk, wv, bv, wo, bo)
    nc = _get_nc()
    trace = os.environ.get("MHA_TRACE", "0") == "1"
    res = run_bass_kernel_spmd(nc, in_maps, core_ids=list(range(N_CORES)), trace=trace)
    LAST_RESULT = res

    out = np.empty((B, S, D), np.float32)
    for b in range(B):
        out[b] = res.results[2 * b]["out"] + res.results[2 * b + 1]["out"]
    return out


# revision 32
# speedup vs baseline: 1.0556x; 1.0556x over previous
"""Causal multi-head attention for TRN2, sharded across 8 NeuronCores.

Problem: x[4,2048,1024] -> 16-head causal self-attention (head_dim 64) with
QKV + output projections, fp32.

Sharding: core c -> batch b = c // 2, head-group g = c % 2 (heads g*8..g*8+7).
Per core: Q/K/V projections use the 512 weight columns of its head-group
(column-parallel); attention runs over its 8 heads; the output projection
uses the matching 512 rows of wo (row-parallel), so each core emits a
partial [2048,1024] output and the host sums the two partials per batch.
bo is added on the g==0 cores only (g==1 cores receive zeros).

v2 over the 263.7us baseline: paired-head row-tiled score matmuls.
The two heads of a QK column block t live in disjoint partition halves
(even head: features on partitions 0:64, odd head: 64:128), so per k-block
the two heads' score matmuls are K=64 row-tiled to array rows 0:63 /
64:127 and run CONCURRENTLY (~2x score throughput), with no extra
copies and no zero-padding of the K contraction. One ACTIVATE covers
both heads' PSUM banks per k-block (same ACT count as the baseline's
kb-pair exp, but with zero junk columns: both slots share the same
causal clip). AV accumulates per head as before; the A PSUM bank is
released early via a [65,512] PSUM->SBUF copy (replaces the baseline's
row-64 sums copy at the same DVE cost), so the 2-bank A pool sustains
back-to-back head pairs.

Retained from the baseline: fp8e4 DoubleRow Q/K projections with
host-scaled x32 weights (score scale folded into the exp scale 2^-13),
bk dropped (softmax-invariant), bq folded into the Q PSUM->SBUF move,
j-chunk-major host layouts, dep-free PE warmup matmuls, bf16 V with a
ones column computing softmax denominators inside the AV matmul, and
the chunk-pipelined filler schedule (chunk j attention interleaved with
chunk j+1 projections and chunk j-1 output projection).

Measured NOT to help (kept out): fp8 E/V/O anywhere in the value path
(numpy-sim rel err 2.8e-2..4e-2 vs the 2e-2 budget); col-tiled AV pairs
(no room for the denominator ones column in 128 array columns).
"""

import os
from contextlib import ExitStack

import numpy as np

import concourse.bacc as bacc
import concourse.mybir as mybir
import concourse.tile as tile
from concourse.bass_utils import run_bass_kernel_spmd
from concourse.masks import make_upper_triangular

F32 = mybir.dt.float32
BF16 = mybir.dt.bfloat16
F8 = mybir.dt.float8e4
AF = mybir.ActivationFunctionType
ALU = mybir.AluOpType
DR = mybir.MatmulPerfMode.DoubleRow

B = 4
S = 2048
D = 1024
HD = 64
HG = 8  # heads per core
QC = HG * HD  # 512 local q/k/v columns
N_CORES = 8
WSCALE = 32.0  # host scale on wq/wk so fp8 stays in normal range
SC = 0.125 / (WSCALE * WSCALE)  # exp scale: 1/sqrt(HD) / (32*32) = 2^-13

_NC_CACHE = {}
LAST_RESULT = None  # BassKernelResults of the most recent kernel() call


def _build_nc(s: int = S, num_devices: int = N_CORES):
    P = 128
    NQ = s // 512  # 512-col q-chunks
    NS = s // P  # 128-row s-tiles
    ND = D // P  # bf16 contraction tiles
    NC = D // 256  # fp8 DoubleRow contraction tiles
    NT = QC // P  # 128-row tiles of the local q/k/v columns
    VW = HD + 1  # 65: per-head V block width (64 cols + ones col)
    VPAD = 7 * VW + P  # 583: last head's lhsT slice must fit

    nc = bacc.Bacc("TRN2", target_bir_lowering=False, debug=False, num_devices=num_devices)

    xf8_d = nc.dram_tensor("xf8", [P, NQ * NC * 2 * 512], F8, kind="ExternalInput").ap()
    xbf_d = nc.dram_tensor("xbf", [P, NQ * ND * 512], BF16, kind="ExternalInput").ap()
    wqf8_d = nc.dram_tensor("wqf8", [P, NC * 2 * QC], F8, kind="ExternalInput").ap()
    wkf8_d = nc.dram_tensor("wkf8", [P, NC * 2 * QC], F8, kind="ExternalInput").ap()
    wv_d = nc.dram_tensor("wv", [P, ND * QC], BF16, kind="ExternalInput").ap()
    wo_d = nc.dram_tensor("wo", [P, NT * D], BF16, kind="ExternalInput").ap()
    bq32_d = nc.dram_tensor("bq32", [QC], F32, kind="ExternalInput").ap()
    bvb_d = nc.dram_tensor("bvb", [P, QC], F32, kind="ExternalInput").ap()
    bob_d = nc.dram_tensor("bob", [P, D], F32, kind="ExternalInput").ap()
    out_d = nc.dram_tensor("out", [s, D], F32, kind="ExternalOutput").ap()

    xf8_r = xf8_d.rearrange("p (j c i s) -> p j c i s", j=NQ, c=NC, i=2)
    xbf_r = xbf_d.rearrange("p (j d s) -> p j d s", j=NQ, d=ND)

    with tile.TileContext(nc) as tc:
        with ExitStack() as ctx:
            consts = ctx.enter_context(tc.tile_pool(name="consts", bufs=1))
            persist = ctx.enter_context(tc.tile_pool(name="persist", bufs=1))
            e_pool = ctx.enter_context(tc.tile_pool(name="epool", bufs=4))
            n_pool = ctx.enter_context(tc.tile_pool(name="npool", bufs=4))
            b_pool = ctx.enter_context(tc.tile_pool(name="bpool", bufs=4))
            acc_pool = ctx.enter_context(tc.tile_pool(name="accpool", bufs=4))
            o_pool = ctx.enter_context(tc.tile_pool(name="opool", bufs=3))
            proj_psum = ctx.enter_context(tc.tile_pool(name="proj_ps", bufs=2, space="PSUM"))
            s_psum = ctx.enter_context(tc.tile_pool(name="s_ps", bufs=2, space="PSUM"))
            a_psum = ctx.enter_context(tc.tile_pool(name="a_ps", bufs=2, space="PSUM"))

            # --- dep-free PE warmup first: matmuls on a memset junk tile
            # open the HAM clock gate while the DMAs stream in
            junk = consts.tile([P, P], BF16)
            nc.gpsimd.memset(junk[:], 0.5)
            warm = s_psum.tile([P, 2, 512], F32, tag="s", name="warm")
            for _ in range(52):
                nc.tensor.matmul(
                    warm[:, 0, 0:P], lhsT=junk[:], rhs=junk[:], start=True, stop=True
                )

            QT = persist.tile([P, NT, s], BF16)
            KT2 = persist.tile([P, NT, s], BF16)  # even head on 0:64, odd on 64:128
            V = persist.tile([P, NS, VPAD + 1], BF16)
            AT = persist.tile([P, NT, s], BF16)

            # V ones columns (denominator trick); V's tail padding stays junk
            # (it only feeds never-read PSUM rows 65+ of the AV accumulators)
            nc.gpsimd.memset(
                V[:, :, 0 : HG * VW].rearrange("p s (h c) -> p s h c", c=VW)[:, :, :, HD : HD + 1],
                1.0,
            )

            tri = consts.tile([P, P], F32)
            make_upper_triangular(nc, tri[:], val=1.0, diag=True)
            tri_b = consts.tile([P, P], BF16)
            nc.vector.tensor_copy(tri_b[:], tri[:])

            # --- inputs in order of first use; DMA issue is ~0.7us each on
            # the sync queue so first-needed must go first
            wqf8_sb = persist.tile([P, NC, 2, QC], F8)
            wkf8_sb = persist.tile([P, NC, 2, QC], F8)
            xf8_sb = persist.tile([P, NQ, NC, 2, 512], F8)
            xbf_sb = persist.tile([P, NQ, ND, 512], BF16)
            wv_sb = persist.tile([P, ND, QC], BF16)
            wo_sb = persist.tile([P, NT, D], BF16)
            bqc = consts.tile([P, NT], F32)
            bvb = consts.tile([P, QC], F32)
            bob = consts.tile([P, D], F32)

            nc.sync.dma_start(wqf8_sb[:], wqf8_d.rearrange("p (c i m) -> p c i m", c=NC, i=2))
            nc.sync.dma_start(xf8_sb[:, 0], xf8_r[:, 0])
            nc.sync.dma_start(wkf8_sb[:], wkf8_d.rearrange("p (c i m) -> p c i m", c=NC, i=2))
            nc.sync.dma_start(bqc[:], bq32_d.rearrange("(t p) -> p t", p=P))
            nc.sync.dma_start(wv_sb[:], wv_d.rearrange("p (d m) -> p d m", d=ND))
            nc.sync.dma_start(xbf_sb[:, 0], xbf_r[:, 0])
            nc.sync.dma_start(bvb[:], bvb_d)
            nc.sync.dma_start(bob[:], bob_d)
            # chunk j's attention interleaves V(j+1) projections in its FIRST
            # filler slots, so xbf[j+1] must land before xf8[j+1]
            for j in range(1, NQ):
                nc.sync.dma_start(xbf_sb[:, j], xbf_r[:, j])
                nc.sync.dma_start(xf8_sb[:, j], xf8_r[:, j])
            nc.sync.dma_start(wo_sb[:], wo_d.rearrange("p (t e) -> p t e", t=NT))

            proj_open = {}  # g -> (ps tile, progress) for half-issued groups

            def proj_group(j, g, half=None):
                """One psum-group of the j-chunk projections; g in 0..11.

                half=0 issues the first half of the matmul chain, half=1 the
                rest + the PSUM read, so a filler slot never blocks the PE
                queue for more than ~4 matmuls. half=None does it all.
                """
                js = slice(j * 512, (j + 1) * 512)
                kind, t = divmod(g, NT)
                if half in (None, 0):
                    ps = proj_psum.tile([P, 512], F32, tag="pp", name="pp")
                    proj_open[(j, g)] = ps
                else:
                    ps = proj_open.pop((j, g))
                if kind == 0:  # Q (fp8 DoubleRow)
                    cr = {None: range(NC), 0: range(NC // 2), 1: range(NC // 2, NC)}[half]
                    for c in cr:
                        nc.tensor.matmul(
                            ps[:],
                            lhsT=wqf8_sb[:, c, :, t * P : (t + 1) * P],
                            rhs=xf8_sb[:, j, c],
                            start=(c == 0),
                            stop=(c == NC - 1),
                            perf_mode=DR,
                        )
                    if half != 0:
                        nc.vector.tensor_scalar_add(QT[:, t, js], ps[:], bqc[:, t : t + 1])
                elif kind == 1:  # K (fp8 DoubleRow)
                    cr = {None: range(NC), 0: range(NC // 2), 1: range(NC // 2, NC)}[half]
                    for c in cr:
                        nc.tensor.matmul(
                            ps[:],
                            lhsT=wkf8_sb[:, c, :, t * P : (t + 1) * P],
                            rhs=xf8_sb[:, j, c],
                            start=(c == 0),
                            stop=(c == NC - 1),
                            perf_mode=DR,
                        )
                    if half != 0:
                        nc.vector.tensor_copy(KT2[:, t, js], ps[:])
                else:  # V s-tile 4j+t (bf16)
                    st = 4 * j + t
                    dr = {None: range(ND), 0: range(ND // 2), 1: range(ND // 2, ND)}[half]
                    for d in dr:
                        nc.tensor.matmul(
                            ps[:],
                            lhsT=xbf_sb[:, j, d, t * P : (t + 1) * P],
                            rhs=wv_sb[:, d, :],
                            start=(d == 0),
                            stop=(d == ND - 1),
                        )
                    if half != 0:
                        dst = V[:, st, 0 : HG * VW].rearrange("p (h c) -> p h c", c=VW)[:, :, 0:HD]
                        src = ps.rearrange("p (h c) -> p h c", c=HD)
                        bsrc = bvb.rearrange("p (h c) -> p h c", c=HD)
                        nc.vector.tensor_tensor(dst, src, bsrc, ALU.add)

            def normalize(j, t, A_ps, pb, last=False):
                """Divide A by the denominator row and store into AT.

                The sums+acc copies free the A bank early so the 2-buf pool
                sustains back-to-back pairs; the last pair skips the acc copy
                (nothing reuses its banks) to shorten the kernel tail.
                """
                sums = n_pool.tile([1, 512], F32, tag="sums", name="sums")
                nc.vector.tensor_copy(sums[:], A_ps[HD : HD + 1, :])
                if last:
                    acc = A_ps
                else:
                    acc = acc_pool.tile([P, 512], F32, name="acc")
                    nc.vector.tensor_copy(acc[0:HD, :], A_ps[0:HD, :])
                rec = n_pool.tile([1, 512], F32, tag="rec", name="rec")
                nc.vector.reciprocal_approx_fast(rec[:], sums[:])
                bc = b_pool.tile([HD, 512], F32, name="bc")
                nc.gpsimd.partition_broadcast(bc[:], rec[0:1, :])
                nc.vector.tensor_tensor(
                    AT[pb : pb + HD, t, j * 512 : (j + 1) * 512],
                    acc[0:HD, :],
                    bc[:],
                    ALU.mult,
                )

            def attn_chunk(j, filler, last_chunk=False):
                """Attention for all 4 head pairs of q-chunk j as ONE
                continuous round stream (no pipeline drain at pair
                boundaries).

                Per k-block kb of pair t, the even head's score matmul
                (K=64, array rows 0:63) and the odd head's (rows 64:127)
                alternate so consecutive PE instructions target disjoint row
                groups and run concurrently. One exp covers both heads'
                banks. AV runs LAG rounds behind the scores so a blocked AV
                (its exp still running, or its tri-mask queued behind other
                DVE work) never head-of-line-blocks the next scores in the
                in-order PE queue and starves the saturated ACT engine.
                """
                nkb = 4 * j + 4
                LAG = 2
                rounds = [(t, kb) for t in range(NT) for kb in range(nkb)]
                rtot = len(rounds) + LAG
                nfill = len(filler)
                Es = {}
                A = {}
                for r in range(rtot):
                    if r < len(rounds):
                        t, kb = rounds[r]
                        y0 = max(0, P * (kb - 4 * j))
                        st = s_psum.tile([P, 2, 512], F32, tag="s", name="sp")
                        sp = st[:, :, y0:]
                        nc.tensor.matmul(
                            sp[:, 0],
                            lhsT=KT2[0:64, t, kb * P : (kb + 1) * P],
                            rhs=QT[0:64, t, j * 512 + y0 : (j + 1) * 512],
                            start=True,
                            stop=True,
                        )
                        nc.tensor.matmul(
                            sp[:, 1],
                            lhsT=KT2[64:128, t, kb * P : (kb + 1) * P],
                            rhs=QT[64:128, t, j * 512 + y0 : (j + 1) * 512],
                            start=True,
                            stop=True,
                        )
                        E = e_pool.tile([P, 2, 512], BF16, name="E")
                        nc.scalar.activation(E[:, :, y0:], sp, AF.Exp, scale=SC)
                        if kb >= 4 * j:  # diagonal block: causal triangle mask
                            nc.vector.tensor_tensor(
                                E[:, 0, y0 : y0 + P], E[:, 0, y0 : y0 + P], tri_b[:], ALU.mult
                            )
                            nc.vector.tensor_tensor(
                                E[:, 1, y0 : y0 + P], E[:, 1, y0 : y0 + P], tri_b[:], ALU.mult
                            )
                        Es[(t, kb)] = (E, y0)
                    if r >= LAG:
                        t, kb = rounds[r - LAG]
                        E, y0 = Es.pop((t, kb))
                        if kb == 0:
                            A[t] = (
                                a_psum.tile([P, 512], F32, tag="A", name="Ae"),
                                a_psum.tile([P, 512], F32, tag="A", name="Ao"),
                            )
                        for m, A_ps in enumerate(A[t]):
                            nc.tensor.matmul(
                                A_ps[:, y0:],
                                lhsT=V[:, kb, (2 * t + m) * VW : (2 * t + m) * VW + P],
                                rhs=E[:, m, y0:],
                                start=(kb == 0),
                                stop=(kb == nkb - 1),
                            )
                        if kb == nkb - 1:
                            last = last_chunk and t == NT - 1
                            A_e, A_o = A.pop(t)
                            normalize(j, t, A_e, 0, last=last)
                            normalize(j, t, A_o, 64, last=last)
                    k0 = (nfill * r) // rtot
                    k1 = (nfill * (r + 1)) // rtot
                    for kind, jj, g, hf in filler[k0:k1]:
                        if kind == "p":
                            proj_group(jj, g, half=hf)
                        else:
                            out_proj_group(jj, g, half=hf)

            out_open = {}

            def out_proj_group(j, g, half=None, use_s_pool=False):
                st = 4 * j + g // 2
                oc = g % 2
                if half in (None, 0):
                    if use_s_pool:  # tail: rotate through the idle score banks
                        o_ps = s_psum.tile([P, 2, 512], F32, tag="s", name="o_ps")[:, 0]
                    else:
                        o_ps = proj_psum.tile([P, 512], F32, tag="pp", name="o_ps")
                    out_open[(j, g)] = o_ps
                else:
                    o_ps = out_open.pop((j, g))
                tr = {None: range(NT), 0: range(NT // 2), 1: range(NT // 2, NT)}[half]
                for t2 in tr:
                    nc.tensor.matmul(
                        o_ps[:],
                        lhsT=AT[:, t2, st * P : (st + 1) * P],
                        rhs=wo_sb[:, t2, oc * 512 : (oc + 1) * 512],
                        start=(t2 == 0),
                        stop=(t2 == NT - 1),
                    )
                if half == 0:
                    return
                ot = o_pool.tile([P, 512], F32, name="ot")
                nc.vector.tensor_tensor(
                    ot[:], o_ps[:], bob[:, oc * 512 : (oc + 1) * 512], ALU.add
                )
                nc.sync.dma_start(
                    out_d[st * P : (st + 1) * P, oc * 512 : (oc + 1) * 512], ot[:]
                )

            def keep_warm(n):
                kw = s_psum.tile([P, 2, 512], F32, tag="s", name="kw")
                for _ in range(n):
                    nc.tensor.matmul(
                        kw[:, 0, 0:P], lhsT=junk[:], rhs=junk[:], start=True, stop=True
                    )

            # The PE has ~43us more total work than the ACT engine, so the
            # excess is parked where ACT is idle anyway: proj(0) AND proj(1)
            # run densely up front (one warm gap-free PE stream while the
            # early exps can't flow yet), and the remaining proj/out work is
            # distributed across chunks in proportion to each chunk's ACT
            # budget (chunk j hosts ~ACT_j - attn_PE_j of filler). Groups are
            # split into halves so one filler slot never blocks the PE queue
            # for more than ~4 matmuls.
            for g in (0, 4, 8, 9, 10, 11, 1, 5, 2, 6, 3, 7):
                proj_group(0, g)
            for g in (8, 9, 10, 11, 0, 4, 1, 5, 2, 6, 3, 7):
                proj_group(1, g)
            fillers = {
                0: [],
                1: [("p", 2, g) for g in (8, 9, 10, 11, 0, 1, 2, 3, 4, 5, 6, 7)],
                2: [("o", 0, g) for g in range(8)]
                + [("p", 3, g) for g in (8, 9, 10, 11, 0, 1, 2, 3, 4, 5, 6, 7)],
                3: [("o", 1, g) for g in range(8)] + [("o", 2, g) for g in range(8)],
            }
            for j in range(NQ):
                filler = [(k, jj, g, hf) for k, jj, g in fillers[j] for hf in (0, 1)]
                attn_chunk(j, filler, last_chunk=(j == NQ - 1))
                if j == NQ - 1:
                    # bridge the last pair's normalize chain before the tail
                    # so the HAM clock gate stays open for the output proj
                    keep_warm(56)
            # tail: the last chunk's output projection, alternating psum
            # pools so the bias-add/DMA chain never gates the matmuls
            for g in range(8):
                out_proj_group(NQ - 1, g, half=None, use_s_pool=bool(g % 2))

    nc.compile()

    return nc


def _get_nc():
    if "nc" not in _NC_CACHE:
        _NC_CACHE["nc"] = _build_nc()
    return _NC_CACHE["nc"]


def make_in_maps(x, wq, bq, wk, bk, wv, bv, wo, bo, n_cores=N_CORES):
    import ml_dtypes

    bf = ml_dtypes.bfloat16
    f8 = ml_dtypes.float8_e4m3
    P = 128
    NQ = S // 512
    x = np.asarray(x, np.float32)
    wq, wk, wv, wo = (np.asarray(a, np.float32) for a in (wq, wk, wv, wo))
    bq = np.asarray(bq, np.float32)
    bv, bo = np.asarray(bv, np.float32), np.asarray(bo, np.float32)

    # per-batch x layouts (shared by both cores of the batch)
    xf8_b, xbf_b = [], []
    for b in range(B):
        xT = np.ascontiguousarray(x[b].T)  # [D, S]
        # [p, j, c, i, s] = xT[256c+128i+p, 512j+s]
        xf8 = (
            xT.reshape(4, 2, P, NQ, 512)
            .transpose(2, 3, 0, 1, 4)
            .reshape(P, -1)
            .astype(f8)
        )
        # [p, j, d, s] = xT[128d+p, 512j+s]
        xbf = (
            xT.reshape(8, P, NQ, 512).transpose(1, 2, 0, 3).reshape(P, -1).astype(bf)
        )
        xf8_b.append(np.ascontiguousarray(xf8))
        xbf_b.append(np.ascontiguousarray(xbf))

    def wqk_f8(w, cs):
        # [p, c, i, m] = WSCALE * w[256c+128i+p, cs][., m]
        return np.ascontiguousarray(
            (w[:, cs] * WSCALE).reshape(4, 2, P, QC).transpose(2, 0, 1, 3).reshape(P, -1).astype(f8)
        )

    in_maps = []
    for c in range(n_cores):
        b, g = c // 2, c % 2
        cs = slice(g * QC, (g + 1) * QC)
        wv_l = np.ascontiguousarray(
            wv[:, cs].reshape(8, P, QC).transpose(1, 0, 2).reshape(P, -1).astype(bf)
        )
        wo_l = np.ascontiguousarray(
            wo[cs, :].reshape(4, P, D).transpose(1, 0, 2).reshape(P, -1).astype(bf)
        )
        in_maps.append(
            {
                "xf8": xf8_b[b],
                "xbf": xbf_b[b],
                "wqf8": wqk_f8(wq, cs),
                "wkf8": wqk_f8(wk, cs),
                "wv": wv_l,
                "wo": wo_l,
                "bq32": np.ascontiguousarray(bq[cs] * WSCALE),
                "bvb": np.ascontiguousarray(np.broadcast_to(bv[cs], (128, QC))),
                "bob": np.ascontiguousarray(
                    np.broadcast_to(bo if g == 0 else np.zeros_like(bo), (128, D))
                ),
            }
        )
    return in_maps


def kernel(x, wq, bq, wk, bk, wv, bv, wo, bo):
    global LAST_RESULT
    in_maps = make_in_maps(x, wq, bq, wk, bk, wv, bv, wo, bo)
    nc = _get_nc()
    trace = os.environ.get("MHA_TRACE", "0") == "1"
    res = run_bass_kernel_spmd(nc, in_maps, core_ids=list(range(N_CORES)), trace=trace)
    LAST_RESULT = res

    out = np.empty((B, S, D), np.float32)
    for b in range(B):
        out[b] = res.results[2 * b]["out"] + res.results[2 * b + 1]["out"]
    return out


# revision 37
# speedup vs baseline: 1.0727x; 1.0163x over previous
"""Causal multi-head attention for TRN2, sharded across 8 NeuronCores.

Problem: x[4,2048,1024] -> 16-head causal self-attention (head_dim 64) with
QKV + output projections, fp32.

Sharding: core c -> batch b = c // 2, head-group g = c % 2 (heads g*8..g*8+7).
Per core: Q/K/V projections use the 512 weight columns of its head-group
(column-parallel); attention runs over its 8 heads; the output projection
uses the matching 512 rows of wo (row-parallel), so each core emits a
partial [2048,1024] output and the host sums the two partials per batch.
bo is added on the g==0 cores only (g==1 cores receive zeros).

v2 over the 263.7us baseline: paired-head row-tiled score matmuls.
The two heads of a QK column block t live in disjoint partition halves
(even head: features on partitions 0:64, odd head: 64:128), so per k-block
the two heads' score matmuls are K=64 row-tiled to array rows 0:63 /
64:127 and run CONCURRENTLY (~2x score throughput), with no extra
copies and no zero-padding of the K contraction. One ACTIVATE covers
both heads' PSUM banks per k-block (same ACT count as the baseline's
kb-pair exp, but with zero junk columns: both slots share the same
causal clip). AV accumulates per head as before; the A PSUM bank is
released early via a [65,512] PSUM->SBUF copy (replaces the baseline's
row-64 sums copy at the same DVE cost), so the 2-bank A pool sustains
back-to-back head pairs.

Retained from the baseline: fp8e4 DoubleRow Q/K projections with
host-scaled x32 weights (score scale folded into the exp scale 2^-13),
bk dropped (softmax-invariant), bq folded into the Q PSUM->SBUF move,
j-chunk-major host layouts, dep-free PE warmup matmuls, bf16 V with a
ones column computing softmax denominators inside the AV matmul, and
the chunk-pipelined filler schedule (chunk j attention interleaved with
chunk j+1 projections and chunk j-1 output projection).

Measured NOT to help (kept out): fp8 E/V/O anywhere in the value path
(numpy-sim rel err 2.8e-2..4e-2 vs the 2e-2 budget); col-tiled AV pairs
(no room for the denominator ones column in 128 array columns).
"""

import os
from contextlib import ExitStack

import numpy as np

import concourse.bacc as bacc
import concourse.mybir as mybir
import concourse.tile as tile
from concourse.bass_utils import run_bass_kernel_spmd
from concourse.masks import make_upper_triangular

F32 = mybir.dt.float32
BF16 = mybir.dt.bfloat16
F8 = mybir.dt.float8e4
AF = mybir.ActivationFunctionType
ALU = mybir.AluOpType
DR = mybir.MatmulPerfMode.DoubleRow

B = 4
S = 2048
D = 1024
HD = 64
HG = 8  # heads per core
QC = HG * HD  # 512 local q/k/v columns
N_CORES = 8
WSCALE = 32.0  # host scale on wq/wk so fp8 stays in normal range
SC = 0.125 / (WSCALE * WSCALE)  # exp scale: 1/sqrt(HD) / (32*32) = 2^-13

_NC_CACHE = {}
LAST_RESULT = None  # BassKernelResults of the most recent kernel() call


def _build_nc(s: int = S, num_devices: int = N_CORES):
    P = 128
    NQ = s // 512  # 512-col q-chunks
    NS = s // P  # 128-row s-tiles
    ND = D // P  # bf16 contraction tiles
    NC = D // 256  # fp8 DoubleRow contraction tiles
    NT = QC // P  # 128-row tiles of the local q/k/v columns
    VW = HD + 1  # 65: per-head V block width (64 cols + ones col)
    VPAD = 7 * VW + P  # 583: last head's lhsT slice must fit

    nc = bacc.Bacc("TRN2", target_bir_lowering=False, debug=False, num_devices=num_devices)

    xf8_d = nc.dram_tensor("xf8", [P, NQ * NC * 2 * 512], F8, kind="ExternalInput").ap()
    xbf_d = nc.dram_tensor("xbf", [P, NQ * ND * 512], BF16, kind="ExternalInput").ap()
    wqf8_d = nc.dram_tensor("wqf8", [P, NC * 2 * QC], F8, kind="ExternalInput").ap()
    wkf8_d = nc.dram_tensor("wkf8", [P, NC * 2 * QC], F8, kind="ExternalInput").ap()
    wv_d = nc.dram_tensor("wv", [P, ND * QC], BF16, kind="ExternalInput").ap()
    wo_d = nc.dram_tensor("wo", [P, NT * D], BF16, kind="ExternalInput").ap()
    bq32_d = nc.dram_tensor("bq32", [QC], F32, kind="ExternalInput").ap()
    bvb_d = nc.dram_tensor("bvb", [P, QC], F32, kind="ExternalInput").ap()
    bob_d = nc.dram_tensor("bob", [P, D], F32, kind="ExternalInput").ap()
    out_d = nc.dram_tensor("out", [s, D], F32, kind="ExternalOutput").ap()

    xf8_r = xf8_d.rearrange("p (j c i s) -> p j c i s", j=NQ, c=NC, i=2)
    xbf_r = xbf_d.rearrange("p (j d s) -> p j d s", j=NQ, d=ND)

    with tile.TileContext(nc) as tc:
        with ExitStack() as ctx:
            consts = ctx.enter_context(tc.tile_pool(name="consts", bufs=1))
            persist = ctx.enter_context(tc.tile_pool(name="persist", bufs=1))
            e_pool = ctx.enter_context(tc.tile_pool(name="epool", bufs=4))
            n_pool = ctx.enter_context(tc.tile_pool(name="npool", bufs=4))
            b_pool = ctx.enter_context(tc.tile_pool(name="bpool", bufs=4))
            acc_pool = ctx.enter_context(tc.tile_pool(name="accpool", bufs=4))
            o_pool = ctx.enter_context(tc.tile_pool(name="opool", bufs=3))
            proj_psum = ctx.enter_context(tc.tile_pool(name="proj_ps", bufs=2, space="PSUM"))
            s_psum = ctx.enter_context(tc.tile_pool(name="s_ps", bufs=2, space="PSUM"))
            a_psum = ctx.enter_context(tc.tile_pool(name="a_ps", bufs=2, space="PSUM"))

            # --- dep-free PE warmup first: matmuls on a memset junk tile
            # open the HAM clock gate while the DMAs stream in
            junk = consts.tile([P, P], BF16)
            nc.gpsimd.memset(junk[:], 0.5)
            warm = s_psum.tile([P, 2, 512], F32, tag="s", name="warm")
            for _ in range(52):
                nc.tensor.matmul(
                    warm[:, 0, 0:P], lhsT=junk[:], rhs=junk[:], start=True, stop=True
                )

            QT = persist.tile([P, NT, s], BF16)
            KT2 = persist.tile([P, NT, s], BF16)  # even head on 0:64, odd on 64:128
            V = persist.tile([P, NS, VPAD + 1], BF16)
            AT = persist.tile([P, NT, s], BF16)

            # V ones columns (denominator trick); V's tail padding stays junk
            # (it only feeds never-read PSUM rows 65+ of the AV accumulators)
            nc.gpsimd.memset(
                V[:, :, 0 : HG * VW].rearrange("p s (h c) -> p s h c", c=VW)[:, :, :, HD : HD + 1],
                1.0,
            )

            tri = consts.tile([P, P], F32)
            make_upper_triangular(nc, tri[:], val=1.0, diag=True)
            tri_b = consts.tile([P, P], BF16)
            nc.vector.tensor_copy(tri_b[:], tri[:])

            # --- inputs in order of first use; DMA issue is ~0.7us each on
            # the sync queue so first-needed must go first
            wqf8_sb = persist.tile([P, NC, 2, QC], F8)
            wkf8_sb = persist.tile([P, NC, 2, QC], F8)
            xf8_sb = persist.tile([P, NQ, NC, 2, 512], F8)
            xbf_sb = persist.tile([P, NQ, ND, 512], BF16)
            wv_sb = persist.tile([P, ND, QC], BF16)
            wo_sb = persist.tile([P, NT, D], BF16)
            bqc = consts.tile([P, NT], F32)
            bvb = consts.tile([P, QC], F32)
            bob = consts.tile([P, D], F32)

            nc.sync.dma_start(wqf8_sb[:], wqf8_d.rearrange("p (c i m) -> p c i m", c=NC, i=2))
            nc.sync.dma_start(xf8_sb[:, 0], xf8_r[:, 0])
            nc.sync.dma_start(wkf8_sb[:], wkf8_d.rearrange("p (c i m) -> p c i m", c=NC, i=2))
            nc.sync.dma_start(bqc[:], bq32_d.rearrange("(t p) -> p t", p=P))
            if NQ > 1:
                nc.sync.dma_start(xf8_sb[:, 1], xf8_r[:, 1])
            nc.sync.dma_start(wv_sb[:], wv_d.rearrange("p (d m) -> p d m", d=ND))
            nc.sync.dma_start(xbf_sb[:, 0], xbf_r[:, 0])
            nc.sync.dma_start(bvb[:], bvb_d)
            if NQ > 1:
                nc.sync.dma_start(xbf_sb[:, 1], xbf_r[:, 1])
            nc.sync.dma_start(bob[:], bob_d)
            # chunk j's attention interleaves V(j+1) projections in its FIRST
            # filler slots, so xbf[j+1] must land before xf8[j+1]
            for j in range(2, NQ):
                nc.sync.dma_start(xbf_sb[:, j], xbf_r[:, j])
                nc.sync.dma_start(xf8_sb[:, j], xf8_r[:, j])
            nc.sync.dma_start(wo_sb[:], wo_d.rearrange("p (t e) -> p t e", t=NT))

            proj_open = {}  # g -> (ps tile, progress) for half-issued groups

            def proj_group(j, g, half=None):
                """One psum-group of the j-chunk projections; g in 0..11.

                half=0 issues the first half of the matmul chain, half=1 the
                rest + the PSUM read, so a filler slot never blocks the PE
                queue for more than ~4 matmuls. half=None does it all.
                """
                js = slice(j * 512, (j + 1) * 512)
                kind, t = divmod(g, NT)
                if half in (None, 0):
                    ps = proj_psum.tile([P, 512], F32, tag="pp", name="pp")
                    proj_open[(j, g)] = ps
                else:
                    ps = proj_open.pop((j, g))
                if kind == 0:  # Q (fp8 DoubleRow)
                    cr = {None: range(NC), 0: range(NC // 2), 1: range(NC // 2, NC)}[half]
                    for c in cr:
                        nc.tensor.matmul(
                            ps[:],
                            lhsT=wqf8_sb[:, c, :, t * P : (t + 1) * P],
                            rhs=xf8_sb[:, j, c],
                            start=(c == 0),
                            stop=(c == NC - 1),
                            perf_mode=DR,
                        )
                    if half != 0:
                        nc.vector.tensor_scalar_add(QT[:, t, js], ps[:], bqc[:, t : t + 1])
                elif kind == 1:  # K (fp8 DoubleRow)
                    cr = {None: range(NC), 0: range(NC // 2), 1: range(NC // 2, NC)}[half]
                    for c in cr:
                        nc.tensor.matmul(
                            ps[:],
                            lhsT=wkf8_sb[:, c, :, t * P : (t + 1) * P],
                            rhs=xf8_sb[:, j, c],
                            start=(c == 0),
                            stop=(c == NC - 1),
                            perf_mode=DR,
                        )
                    if half != 0:
                        nc.vector.tensor_copy(KT2[:, t, js], ps[:])
                else:  # V s-tile 4j+t (bf16)
                    st = 4 * j + t
                    dr = {None: range(ND), 0: range(ND // 2), 1: range(ND // 2, ND)}[half]
                    for d in dr:
                        nc.tensor.matmul(
                            ps[:],
                            lhsT=xbf_sb[:, j, d, t * P : (t + 1) * P],
                            rhs=wv_sb[:, d, :],
                            start=(d == 0),
                            stop=(d == ND - 1),
                        )
                    if half != 0:
                        dst = V[:, st, 0 : HG * VW].rearrange("p (h c) -> p h c", c=VW)[:, :, 0:HD]
                        src = ps.rearrange("p (h c) -> p h c", c=HD)
                        bsrc = bvb.rearrange("p (h c) -> p h c", c=HD)
                        nc.vector.tensor_tensor(dst, src, bsrc, ALU.add)

            def normalize(j, t, A_ps, pb, last=False):
                """Divide A by the denominator row and store into AT.

                The sums+acc copies free the A bank early so the 2-buf pool
                sustains back-to-back pairs; the last pair skips the acc copy
                (nothing reuses its banks) to shorten the kernel tail.
                """
                sums = n_pool.tile([1, 512], F32, tag="sums", name="sums")
                nc.vector.tensor_copy(sums[:], A_ps[HD : HD + 1, :])
                if last:
                    acc = A_ps
                else:
                    acc = acc_pool.tile([P, 512], F32, name="acc")
                    nc.vector.tensor_copy(acc[0:HD, :], A_ps[0:HD, :])
                rec = n_pool.tile([1, 512], F32, tag="rec", name="rec")
                nc.vector.reciprocal_approx_fast(rec[:], sums[:])
                bc = b_pool.tile([HD, 512], F32, name="bc")
                nc.gpsimd.partition_broadcast(bc[:], rec[0:1, :])
                nc.vector.tensor_tensor(
                    AT[pb : pb + HD, t, j * 512 : (j + 1) * 512],
                    acc[0:HD, :],
                    bc[:],
                    ALU.mult,
                )

            def attn_chunk(j, filler, last_chunk=False):
                """Attention for all 4 head pairs of q-chunk j as ONE
                continuous round stream (no pipeline drain at pair
                boundaries).

                Per k-block kb of pair t, the even head's score matmul
                (K=64, array rows 0:63) and the odd head's (rows 64:127)
                alternate so consecutive PE instructions target disjoint row
                groups and run concurrently. One exp covers both heads'
                banks. AV runs LAG rounds behind the scores so a blocked AV
                (its exp still running, or its tri-mask queued behind other
                DVE work) never head-of-line-blocks the next scores in the
                in-order PE queue and starves the saturated ACT engine.
                """
                nkb = 4 * j + 4
                LAG = 2
                rounds = [(t, kb) for t in range(NT) for kb in range(nkb)]
                rtot = len(rounds) + LAG
                nfill = len(filler)
                Es = {}
                A = {}
                for r in range(rtot):
                    if r < len(rounds):
                        t, kb = rounds[r]
                        y0 = max(0, P * (kb - 4 * j))
                        st = s_psum.tile([P, 2, 512], F32, tag="s", name="sp")
                        sp = st[:, :, y0:]
                        nc.tensor.matmul(
                            sp[:, 0],
                            lhsT=KT2[0:64, t, kb * P : (kb + 1) * P],
                            rhs=QT[0:64, t, j * 512 + y0 : (j + 1) * 512],
                            start=True,
                            stop=True,
                        )
                        nc.tensor.matmul(
                            sp[:, 1],
                            lhsT=KT2[64:128, t, kb * P : (kb + 1) * P],
                            rhs=QT[64:128, t, j * 512 + y0 : (j + 1) * 512],
                            start=True,
                            stop=True,
                        )
                        E = e_pool.tile([P, 2, 512], BF16, name="E")
                        nc.scalar.activation(E[:, :, y0:], sp, AF.Exp, scale=SC)
                        if kb >= 4 * j:  # diagonal block: causal triangle mask
                            nc.vector.tensor_tensor(
                                E[:, 0, y0 : y0 + P], E[:, 0, y0 : y0 + P], tri_b[:], ALU.mult
                            )
                            nc.vector.tensor_tensor(
                                E[:, 1, y0 : y0 + P], E[:, 1, y0 : y0 + P], tri_b[:], ALU.mult
                            )
                        Es[(t, kb)] = (E, y0)
                    if r >= LAG:
                        t, kb = rounds[r - LAG]
                        E, y0 = Es.pop((t, kb))
                        if kb == 0:
                            A[t] = (
                                a_psum.tile([P, 512], F32, tag="A", name="Ae"),
                                a_psum.tile([P, 512], F32, tag="A", name="Ao"),
                            )
                        for m, A_ps in enumerate(A[t]):
                            nc.tensor.matmul(
                                A_ps[:, y0:],
                                lhsT=V[:, kb, (2 * t + m) * VW : (2 * t + m) * VW + P],
                                rhs=E[:, m, y0:],
                                start=(kb == 0),
                                stop=(kb == nkb - 1),
                            )
                        if kb == nkb - 1:
                            last = last_chunk and t == NT - 1
                            A_e, A_o = A.pop(t)
                            normalize(j, t, A_e, 0, last=last)
                            normalize(j, t, A_o, 64, last=last)
                    k0 = (nfill * r) // rtot
                    k1 = (nfill * (r + 1)) // rtot
                    for kind, jj, g, hf in filler[k0:k1]:
                        if kind == "p":
                            proj_group(jj, g, half=hf)
                        else:
                            out_proj_group(jj, g, half=hf)

            out_open = {}

            def out_proj_group(j, g, half=None, use_s_pool=False):
                st = 4 * j + g // 2
                oc = g % 2
                if half in (None, 0):
                    if use_s_pool:  # tail: rotate through the idle score banks
                        o_ps = s_psum.tile([P, 2, 512], F32, tag="s", name="o_ps")[:, 0]
                    else:
                        o_ps = proj_psum.tile([P, 512], F32, tag="pp", name="o_ps")
                    out_open[(j, g)] = o_ps
                else:
                    o_ps = out_open.pop((j, g))
                tr = {None: range(NT), 0: range(NT // 2), 1: range(NT // 2, NT)}[half]
                for t2 in tr:
                    nc.tensor.matmul(
                        o_ps[:],
                        lhsT=AT[:, t2, st * P : (st + 1) * P],
                        rhs=wo_sb[:, t2, oc * 512 : (oc + 1) * 512],
                        start=(t2 == 0),
                        stop=(t2 == NT - 1),
                    )
                if half == 0:
                    return
                ot = o_pool.tile([P, 512], F32, name="ot")
                nc.vector.tensor_tensor(
                    ot[:], o_ps[:], bob[:, oc * 512 : (oc + 1) * 512], ALU.add
                )
                nc.sync.dma_start(
                    out_d[st * P : (st + 1) * P, oc * 512 : (oc + 1) * 512], ot[:]
                )

            def keep_warm(n):
                kw = s_psum.tile([P, 2, 512], F32, tag="s", name="kw")
                for _ in range(n):
                    nc.tensor.matmul(
                        kw[:, 0, 0:P], lhsT=junk[:], rhs=junk[:], start=True, stop=True
                    )

            # The PE has ~43us more total work than the ACT engine, so the
            # excess is parked where ACT is idle anyway: proj(0) AND proj(1)
            # run densely up front (one warm gap-free PE stream while the
            # early exps can't flow yet), and the remaining proj/out work is
            # distributed across chunks in proportion to each chunk's ACT
            # budget (chunk j hosts ~ACT_j - attn_PE_j of filler). Groups are
            # split into halves so one filler slot never blocks the PE queue
            # for more than ~4 matmuls.
            vqk = (8, 9, 10, 11, 0, 1, 2, 3, 4, 5, 6, 7)
            # Q/K groups first: they only need the small early x/w DMAs, so
            # the PE stream stays dense (and the HAM clock gate open) while
            # the larger xbf transfers for the V groups are still in flight
            for g in (0, 4, 1, 5, 2, 6, 3, 7, 8, 9, 10, 11):
                proj_group(0, g)
            if NQ > 1:
                for g in (0, 4, 1, 5, 2, 6, 3, 7, 8, 9, 10, 11):
                    proj_group(1, g)
            if NQ == 4:
                fillers = {
                    0: [],
                    1: [("p", 2, g) for g in vqk],
                    2: [("o", 0, g) for g in range(8)] + [("p", 3, g) for g in vqk],
                    3: [("o", 1, g) for g in range(8)] + [("o", 2, g) for g in range(8)],
                }
            else:  # generic fallback for debug sizes
                fillers = {j: [] for j in range(NQ)}
                for jj in range(2, NQ):  # proj(jj) during chunk jj-1
                    fillers[jj - 1] += [("p", jj, g) for g in vqk]
                for jj in range(0, NQ - 1):  # out(jj) during chunk jj+1..NQ-1
                    fillers[min(jj + 1, NQ - 1)] += [("o", jj, g) for g in range(8)]
            for j in range(NQ):
                filler = [(k, jj, g, hf) for k, jj, g in fillers[j] for hf in (0, 1)]
                attn_chunk(j, filler, last_chunk=(j == NQ - 1))
                if j == NQ - 1:
                    # bridge the last pair's normalize chain before the tail
                    # so the HAM clock gate stays open for the output proj
                    keep_warm(56)
            # tail: the last chunk's output projection, alternating psum
            # pools so the bias-add/DMA chain never gates the matmuls
            for g in range(8):
                out_proj_group(NQ - 1, g, half=None, use_s_pool=bool(g % 2))

    nc.compile()

    return nc


def _get_nc():
    if "nc" not in _NC_CACHE:
        _NC_CACHE["nc"] = _build_nc()
    return _NC_CACHE["nc"]


def make_in_maps(x, wq, bq, wk, bk, wv, bv, wo, bo, n_cores=N_CORES):
    import ml_dtypes

    bf = ml_dtypes.bfloat16
    f8 = ml_dtypes.float8_e4m3
    P = 128
    NQ = S // 512
    x = np.asarray(x, np.float32)
    wq, wk, wv, wo = (np.asarray(a, np.float32) for a in (wq, wk, wv, wo))
    bq = np.asarray(bq, np.float32)
    bv, bo = np.asarray(bv, np.float32), np.asarray(bo, np.float32)

    # per-batch x layouts (shared by both cores of the batch)
    xf8_b, xbf_b = [], []
    for b in range(B):
        xT = np.ascontiguousarray(x[b].T)  # [D, S]
        # [p, j, c, i, s] = xT[256c+128i+p, 512j+s]
        xf8 = (
            xT.reshape(4, 2, P, NQ, 512)
            .transpose(2, 3, 0, 1, 4)
            .reshape(P, -1)
            .astype(f8)
        )
        # [p, j, d, s] = xT[128d+p, 512j+s]
        xbf = (
            xT.reshape(8, P, NQ, 512).transpose(1, 2, 0, 3).reshape(P, -1).astype(bf)
        )
        xf8_b.append(np.ascontiguousarray(xf8))
        xbf_b.append(np.ascontiguousarray(xbf))

    def wqk_f8(w, cs):
        # [p, c, i, m] = WSCALE * w[256c+128i+p, cs][., m]
        return np.ascontiguousarray(
            (w[:, cs] * WSCALE).reshape(4, 2, P, QC).transpose(2, 0, 1, 3).reshape(P, -1).astype(f8)
        )

    in_maps = []
    for c in range(n_cores):
        b, g = c // 2, c % 2
        cs = slice(g * QC, (g + 1) * QC)
        wv_l = np.ascontiguousarray(
            wv[:, cs].reshape(8, P, QC).transpose(1, 0, 2).reshape(P, -1).astype(bf)
        )
        wo_l = np.ascontiguousarray(
            wo[cs, :].reshape(4, P, D).transpose(1, 0, 2).reshape(P, -1).astype(bf)
        )
        in_maps.append(
            {
                "xf8": xf8_b[b],
                "xbf": xbf_b[b],
                "wqf8": wqk_f8(wq, cs),
                "wkf8": wqk_f8(wk, cs),
                "wv": wv_l,
                "wo": wo_l,
                "bq32": np.ascontiguousarray(bq[cs] * WSCALE),
                "bvb": np.ascontiguousarray(np.broadcast_to(bv[cs], (128, QC))),
                "bob": np.ascontiguousarray(
                    np.broadcast_to(bo if g == 0 else np.zeros_like(bo), (128, D))
                ),
            }
        )
    return in_maps


def kernel(x, wq, bq, wk, bk, wv, bv, wo, bo):
    global LAST_RESULT
    in_maps = make_in_maps(x, wq, bq, wk, bk, wv, bv, wo, bo)
    nc = _get_nc()
    trace = os.environ.get("MHA_TRACE", "0") == "1"
    res = run_bass_kernel_spmd(nc, in_maps, core_ids=list(range(N_CORES)), trace=trace)
    LAST_RESULT = res

    out = np.empty((B, S, D), np.float32)
    for b in range(B):
        out[b] = res.results[2 * b]["out"] + res.results[2 * b + 1]["out"]
    return out
